# revision 12
# baseline (speedup 1.0000x reference)
"""BinaryLinear Trainium2 kernel.

Computes: out = binarize(x) @ binarize(weight - threshold).T * 2^round(clip(shift, -8, 0))

where binarize(v) = +1 if v >= 0 else -1, over x [B,S,IN], weight [OUT,IN].

Strategy (8 NeuronCores, tensor-parallel over OUT):
  - each core gets the full x and a 2048-row slice of weight/threshold
  - host prep is exact for this computation: the device only ever uses
    the SIGN of x and of (weight - threshold), so the host binarizes both
    to +/-1.0 in fp8e4m3 (a byte select of 0x38/0xB8 -- no rounding
    anywhere; the compare (w - thr) >= 0 is evaluated in f32 exactly as
    the reference does).  The host also lays both out so every device DMA
    is a plain contiguous pull:
      * x is pair-interleave-transposed to [IN/2, 2*B*S]: row c, col
        2s+j holds sign(x[s, 2c+j]) -- a contiguous DMA lands the packed
        DoubleRow moving layout [p, (s j)] (pair bytes adjacent), with
        contraction row k = 256g + 2p + j for slab g
      * w is pre-shaped into the stationary slab layout [P, pass, t, o]
        (t = 2g+j, 16B-aligned pair stride) so slab chunks are 2 KB-per-
        partition contiguous DMAs
  - on device the kernel is pure dataflow: DMA in, 4096 fp8 DoubleRow
    matmuls (256 contraction rows each, 2x PE rate) accumulating into
    fp32 PSUM, evict each bank to bf16 with the pow2 scale folded in
    (|out| <= 4096, scale <= 1 -> bf16 exact to 2^-9 relative, exact
    when |out*scale| < 2^9 quantum), store.  Host upcasts to f32.

Pipeline (from trace analysis: the PE streams at ~216-220 ns/matmul
whenever fed; the schedule exists to keep it fed from the first
microsecond to the last):
  - x super-chunks of 1024 s-columns (2 psum-chunks), 16 slabs each;
    super 0's slabs are split into half-slab (512-col) DMAs and pass 0
    is emitted g-major across all 8 psum banks, so the first matmul
    needs only ~256 KB of HBM (one x half-slab + one w chunk) and the
    PE consumption rate per slab (8 matmuls/half) stays above the HBM
    arrival rate from t~=7 us on -- no wait-for-full-super startup stall
  - w rides the scalar HWDGE (32 chunk DMAs, pass-major), x and output
    stores ride the sync HWDGE; nothing else queues there, so startup
    HBM is consumed by exactly the two critical streams
  - from super 1 on, accumulation is bank-major (for each o-block: all
    16 k-groups, then evict that bank immediately) so evictions overlap
    the next bank's matmul chain and the kernel tail is one bank deep;
    the next super's x slabs are emitted one per bank group, each AFTER
    that group's evict+store, so prefetch never sits ahead of the
    eviction chain in the Vector FIFO
  - psum: 4 tags x 2 bufs = all 8 banks; evictions on the DVE (the only
    Vector-engine work in the kernel)
"""

import sys

if "/opt/trn_rl_repo" not in sys.path:
    sys.path.insert(0, "/opt/trn_rl_repo")

import numpy as np

B, S, IN, OUT = 4, 2048, 4096, 16384
N_CORES = 8
O_SHARD = OUT // N_CORES  # 2048
P = 128  # partitions
N_CH = 512  # psum free-dim chunk (one bank of fp32)
SUP = 2  # s-chunks per x super-chunk

# fp8e4m3 byte encodings of +1.0 / -1.0 (identical in OCP e4m3fn and
# TRN FP8_EXP4: both use bias 7 and agree on all values up to +-240)
_FP8_ONE = 0x38
_FP8_NEG_ONE = 0xB8

# dev knobs (test.py only; harness uses defaults)
_TRACE = False
_LAST_RESULTS = None


def build_program(s_rows=B * S, o_shard=O_SHARD, kdim=IN, scale=1.0):
    """Trace the single-core SPMD program.

    Inputs: x [kdim//2, 2*s_rows] fp8 (host pair-interleaved transpose of
    sign(x): row c, column 2s+j holds sign(x[s, 2c+j])), w [P, n_pass *
    n_kt * 4P] fp8 (host-preshaped stationary slabs; per partition p the
    free index (ps, t, o) holds sign(w - thr) at out-row ps*4P + o,
    contraction k = 256*(t//2) + 2p + (t%2)).
    Output: outT [o_shard, s_rows] bf16.
    """
    import concourse.mybir as mybir
    import concourse.tile as tile
    from concourse import bacc
    from concourse.alu_op_type import AluOpType

    f32 = mybir.dt.float32
    bf16 = mybir.dt.bfloat16
    fp8 = mybir.dt.float8e4
    DR = mybir.MatmulPerfMode.DoubleRow

    n_g = kdim // 256      # DoubleRow groups (256 contraction rows each)
    n_kt = kdim // P       # 128-row k-tiles in the stationary slab
    n_ob = o_shard // P    # o-blocks of 128
    n_pass = n_ob // 4     # 4 o-blocks (psum banks) per pass
    S_SUP = SUP * N_CH     # s-columns per x super-chunk
    n_sup = s_rows // S_SUP
    MC = min(4, n_kt)      # k-tiles per w load chunk
    n_mc = n_kt // MC
    W_PASS = n_kt * 4 * P  # w elements per pass per partition
    assert s_rows % S_SUP == 0 and o_shard % (4 * P) == 0 and kdim % 256 == 0
    assert n_kt % MC == 0

    nc = bacc.Bacc(None, target_bir_lowering=False, debug=False)

    x_d = nc.dram_tensor("x", [kdim // 2, 2 * s_rows], fp8,
                         kind="ExternalInput")
    w_d = nc.dram_tensor("w", [P, n_pass * W_PASS], fp8,
                         kind="ExternalInput")
    o_d = nc.dram_tensor("outT", [o_shard, s_rows], bf16,
                         kind="ExternalOutput")

    with tile.TileContext(nc) as tc:
        with (
            tc.tile_pool(name="xs", bufs=2) as xs_pool,
            tc.tile_pool(name="w8", bufs=1) as w8_pool,
            tc.tile_pool(name="outp", bufs=10) as out_pool,
            tc.tile_pool(name="ps", bufs=2, space="PSUM") as ps_pool,
        ):
            wslabs = [
                w8_pool.tile([P, n_kt, 4 * P], fp8, name=f"wslab{ps}",
                             tag=f"wslab{ps}")
                for ps in range(n_pass)
            ]

            def emit_wchunk(ps, t0, nt):
                # nt k-tiles [t0, t0+nt) of this pass's o-range, contiguous
                # per partition, straight into the stationary slab
                dst = wslabs[ps][:, t0:t0 + nt, :]
                c0 = ps * W_PASS + t0 * 4 * P
                src = w_d[:, c0:c0 + nt * 4 * P]
                nc.scalar.dma_start(dst, src.rearrange("p (t o) -> p t o",
                                                       t=nt))

            def emit_xslab(xsup, u, g):
                # one packed pair slab: contiguous DMA, 2 KB/partition
                c0 = 2 * u * S_SUP
                nc.sync.dma_start(
                    xsup[:, g, :], x_d[g * P:(g + 1) * P, c0:c0 + 2 * S_SUP])

            def emit_xhalf(xsup, u, g, l):
                # half-slab (one s-chunk's worth): startup granularity
                c0 = 2 * u * S_SUP + 2 * l * N_CH
                nc.sync.dma_start(
                    xsup[:, g, 2 * l * N_CH:2 * (l + 1) * N_CH],
                    x_d[g * P:(g + 1) * P, c0:c0 + 2 * N_CH])

            def new_xsup():
                return xs_pool.tile([P, n_g, 2 * S_SUP], fp8, name="xsup",
                                    tag="xs")

            def rhs_ap(xsup, g, l):
                return xsup[:, g,
                            2 * l * N_CH:2 * (l + 1) * N_CH].rearrange(
                    "p (s j) -> p j s", j=2)

            def emit_evict(pst, ob, sc):
                # evict one bank to bf16 with the pow2 scale folded in;
                # store over the sync HWDGE
                ot = out_pool.tile([P, N_CH], bf16, name="ot", tag="ot")
                nc.vector.tensor_scalar(ot[:], pst[:], float(scale), None,
                                        AluOpType.mult)
                nc.sync.dma_start(
                    o_d[ob * P:(ob + 1) * P,
                        sc * N_CH:(sc + 1) * N_CH], ot[:])

            def bank_group(xsup, l, sc, ps, i):
                # accumulate one o-block over all k-groups, then evict
                pst = ps_pool.tile([P, N_CH], f32, name=f"ps{i}",
                                   tag=f"ps{i}")
                for g in range(n_g):
                    nc.tensor.matmul(
                        pst[:],
                        wslabs[ps][:, 2 * g:2 * g + 2, i * P:(i + 1) * P],
                        rhs_ap(xsup, g, l),
                        start=(g == 0), stop=(g == n_g - 1),
                        perf_mode=DR)
                emit_evict(pst, ps * 4 + i, sc)

            # --- startup: super 0's x goes out as 32 half-slab DMAs on
            # the sync queue; all w chunks go out pass-major on the
            # scalar queue.  The queues drain concurrently, so the first
            # matmul needs only the first half-slab + first (quarter) w
            # chunk ---
            xsup0 = new_xsup()
            for g in range(n_g):
                for l in range(SUP):
                    emit_xhalf(xsup0, 0, g, l)
            emit_wchunk(0, 0, 2)  # split chunk 0: first matmul needs t 0..1
            emit_wchunk(0, 2, 2)
            for ps in range(n_pass):
                for mc in range(n_mc):
                    if ps == 0 and mc == 0:
                        continue
                    emit_wchunk(ps, mc * MC, MC)

            # (A PE warm-up against the p-state clock ramp was tried --
            # full-size DR matmuls on scratch during the DMA wait DO
            # pre-ramp the clock, tiny ones don't -- but it measurably
            # buys nothing: the ramp already overlaps the HBM-arrival-
            # limited first microseconds, and starting real matmuls at
            # full speed just converts ramp time into x-arrival stalls.)

            # --- super 0, pass 0: g-major across all 8 psum banks so PE
            # demand tracks the x half-slab arrival order (consumption
            # is 8 matmuls per half-slab; arrival is faster from t~=7us)
            psts0 = {}
            for l in range(SUP):
                for i in range(4):
                    psts0[(l, i)] = ps_pool.tile([P, N_CH], f32,
                                                 name=f"ps{i}", tag=f"ps{i}")
            for g in range(n_g):
                for l in range(SUP):
                    rhs = rhs_ap(xsup0, g, l)
                    for i in range(4):
                        nc.tensor.matmul(
                            psts0[(l, i)][:],
                            wslabs[0][:, 2 * g:2 * g + 2, i * P:(i + 1) * P],
                            rhs,
                            start=(g == 0), stop=(g == n_g - 1),
                            perf_mode=DR)
            for l in range(SUP):
                for i in range(4):
                    emit_evict(psts0[(l, i)], i, l)

            # --- super 0, passes 1..3: pass-major bank groups; super 1's
            # x slabs are emitted one per bank group, each AFTER that
            # group's evict+store ---
            xs_next = new_xsup() if n_sup > 1 else None
            pending_g = list(range(n_g)) if xs_next is not None else []

            def after_group(u):
                if pending_g:
                    emit_xslab(xs_next, u + 1, pending_g.pop(0))

            for ps in range(1, n_pass):
                for l in range(SUP):
                    for i in range(4):
                        bank_group(xsup0, l, l, ps, i)
                        after_group(0)

            # --- supers 1..: bank-major, one x prefetch slab per group
            xsup = xs_next
            for u in range(1, n_sup):
                while pending_g:  # catch up if super 0 underfed the list
                    after_group(u - 1)
                xs_next = new_xsup() if u + 1 < n_sup else None
                pending_g = list(range(n_g)) if xs_next is not None else []
                for l in range(SUP):
                    sc = u * SUP + l
                    for ps in range(n_pass):
                        for i in range(4):
                            bank_group(xsup, l, sc, ps, i)
                            after_group(u)
                while pending_g:
                    after_group(u)
                xsup = xs_next

    nc.compile()
    return nc


def _host_scale(shift_param):
    # np.round is round-half-to-even, matching jnp.round
    s = np.clip(np.float64(np.float32(shift_param)), -8.0, 0.0)
    return float(np.exp2(np.round(s)))


def kernel(x, weight, threshold, shift_param):
    import ml_dtypes

    from concourse.bass_utils import run_bass_kernel_spmd

    fp8 = ml_dtypes.float8_e4m3fn
    scale = _host_scale(shift_param)
    nc = build_program(scale=scale)

    # host prep: binarize to +/-1.0 fp8 via byte select (exact -- the
    # compare is the computation; 0x38/0xB8 are the e4m3 encodings of
    # +/-1.0) and lay out for contiguous device DMAs
    xf = np.asarray(x, np.float32).reshape(B * S, IN)
    x8 = np.where(xf >= 0.0, np.uint8(_FP8_ONE), np.uint8(_FP8_NEG_ONE))
    # pair-interleave transpose: [B*S, IN] -> [IN/2, 2*B*S], row c col
    # 2s+j = sign(x[s, 2c+j])
    x8 = np.ascontiguousarray(
        x8.reshape(B * S, IN // 2, 2).transpose(1, 0, 2)
        .reshape(IN // 2, 2 * B * S)).view(fp8)

    thr_f = np.asarray(threshold, np.float32).reshape(OUT, 1)
    d = np.asarray(weight, np.float32) - thr_f  # f32, exactly as reference
    w8 = np.where(d >= 0.0, np.uint8(_FP8_ONE), np.uint8(_FP8_NEG_ONE))

    n_pass = O_SHARD // (4 * P)  # 4
    n_g = IN // 256              # 16
    in_maps = []
    for c in range(N_CORES):
        wc = w8[c * O_SHARD:(c + 1) * O_SHARD]  # [2048, 4096] uint8
        # [o, k] -> [p, ps, g, j, oo]: o = ps*512 + oo, k = 256g + 2p + j
        wcs = wc.reshape(n_pass, 4 * P, n_g, P, 2).transpose(3, 0, 2, 4, 1)
        wcs = np.ascontiguousarray(wcs).reshape(P, -1).view(fp8)
        in_maps.append({"x": x8, "w": wcs})

    res = run_bass_kernel_spmd(nc, in_maps, list(range(N_CORES)),
                               trace=_TRACE)
    global _LAST_RESULTS
    _LAST_RESULTS = res
    shards = [res.results[c]["outT"] for c in range(N_CORES)]
    full_t = np.concatenate(shards, axis=0)  # [OUT, B*S]
    full = np.ascontiguousarray(full_t.astype(np.float32).T).reshape(B, S, OUT)
    return full


# revision 13
# speedup vs baseline: 1.0022x; 1.0022x over previous
"""BinaryLinear Trainium2 kernel.

Computes: out = binarize(x) @ binarize(weight - threshold).T * 2^round(clip(shift, -8, 0))

where binarize(v) = +1 if v >= 0 else -1, over x [B,S,IN], weight [OUT,IN].

Strategy (8 NeuronCores, tensor-parallel over OUT):
  - each core gets the full x and a 2048-row slice of weight/threshold
  - host prep is exact for this computation: the device only ever uses
    the SIGN of x and of (weight - threshold), so the host binarizes both
    to +/-1.0 in fp8e4m3 (a byte select of 0x38/0xB8 -- no rounding
    anywhere; the compare (w - thr) >= 0 is evaluated in f32 exactly as
    the reference does).  The host also lays both out so every device DMA
    is a plain contiguous pull:
      * x is pair-interleave-transposed to [IN/2, 2*B*S]: row c, col
        2s+j holds sign(x[s, 2c+j]) -- a contiguous DMA lands the packed
        DoubleRow moving layout [p, (s j)] (pair bytes adjacent), with
        contraction row k = 256g + 2p + j for slab g
      * w is pre-shaped into the stationary slab layout [P, pass, t, o]
        (t = 2g+j, 16B-aligned pair stride) so slab chunks are 2 KB-per-
        partition contiguous DMAs
  - on device the kernel is pure dataflow: DMA in, 4096 fp8 DoubleRow
    matmuls (256 contraction rows each, 2x PE rate) accumulating into
    fp32 PSUM, evict each bank to bf16 with the pow2 scale folded in
    (|out| <= 4096, scale <= 1 -> bf16 exact to 2^-9 relative, exact
    when |out*scale| < 2^9 quantum), store.  Host upcasts to f32.

Pipeline (from trace analysis: the PE streams at ~216-220 ns/matmul
whenever fed; the schedule exists to keep it fed from the first
microsecond to the last):
  - x super-chunks of 1024 s-columns (2 psum-chunks), 16 slabs each;
    super 0's slabs are split into half-slab (512-col) DMAs and pass 0
    is emitted g-major across all 8 psum banks, so the first matmul
    needs only ~256 KB of HBM (one x half-slab + one w chunk) and the
    PE consumption rate per slab (8 matmuls/half) stays above the HBM
    arrival rate from t~=7 us on -- no wait-for-full-super startup stall
  - w rides the scalar HWDGE (32 chunk DMAs, pass-major), x and output
    stores ride the sync HWDGE; nothing else queues there, so startup
    HBM is consumed by exactly the two critical streams
  - from super 1 on, accumulation is bank-major (for each o-block: all
    16 k-groups, then evict that bank immediately) so evictions overlap
    the next bank's matmul chain and the kernel tail is one bank deep;
    the next super's x slabs are emitted one per bank group, each AFTER
    that group's evict+store, so prefetch never sits ahead of the
    eviction chain in the Vector FIFO
  - psum: 4 tags x 2 bufs = all 8 banks; evictions on the DVE (the only
    Vector-engine work in the kernel)
"""

import sys

if "/opt/trn_rl_repo" not in sys.path:
    sys.path.insert(0, "/opt/trn_rl_repo")

import numpy as np

B, S, IN, OUT = 4, 2048, 4096, 16384
N_CORES = 8
O_SHARD = OUT // N_CORES  # 2048
P = 128  # partitions
N_CH = 512  # psum free-dim chunk (one bank of fp32)
SUP = 2  # s-chunks per x super-chunk

# fp8e4m3 byte encodings of +1.0 / -1.0 (identical in OCP e4m3fn and
# TRN FP8_EXP4: both use bias 7 and agree on all values up to +-240)
_FP8_ONE = 0x38
_FP8_NEG_ONE = 0xB8

# dev knobs (test.py only; harness uses defaults)
_TRACE = False
_LAST_RESULTS = None


def build_program(s_rows=B * S, o_shard=O_SHARD, kdim=IN, scale=1.0):
    """Trace the single-core SPMD program.

    Inputs: x [kdim//2, 2*s_rows] fp8 (host pair-interleaved transpose of
    sign(x): row c, column 2s+j holds sign(x[s, 2c+j])), w [P, n_pass *
    n_kt * 4P] fp8 (host-preshaped stationary slabs; per partition p the
    free index (ps, t, o) holds sign(w - thr) at out-row ps*4P + o,
    contraction k = 256*(t//2) + 2p + (t%2)).
    Output: outT [o_shard, s_rows] bf16.
    """
    import concourse.mybir as mybir
    import concourse.tile as tile
    from concourse import bacc
    from concourse.alu_op_type import AluOpType

    f32 = mybir.dt.float32
    bf16 = mybir.dt.bfloat16
    fp8 = mybir.dt.float8e4
    DR = mybir.MatmulPerfMode.DoubleRow

    n_g = kdim // 256      # DoubleRow groups (256 contraction rows each)
    n_kt = kdim // P       # 128-row k-tiles in the stationary slab
    n_ob = o_shard // P    # o-blocks of 128
    n_pass = n_ob // 4     # 4 o-blocks (psum banks) per pass
    S_SUP = SUP * N_CH     # s-columns per x super-chunk
    n_sup = s_rows // S_SUP
    MC = min(4, n_kt)      # k-tiles per w load chunk
    n_mc = n_kt // MC
    W_PASS = n_kt * 4 * P  # w elements per pass per partition
    assert s_rows % S_SUP == 0 and o_shard % (4 * P) == 0 and kdim % 256 == 0
    assert n_kt % MC == 0

    nc = bacc.Bacc(None, target_bir_lowering=False, debug=False)

    x_d = nc.dram_tensor("x", [kdim // 2, 2 * s_rows], fp8,
                         kind="ExternalInput")
    w_d = nc.dram_tensor("w", [P, n_pass * W_PASS], fp8,
                         kind="ExternalInput")
    o_d = nc.dram_tensor("outT", [o_shard, s_rows], bf16,
                         kind="ExternalOutput")

    with tile.TileContext(nc) as tc:
        with (
            tc.tile_pool(name="xs", bufs=2) as xs_pool,
            tc.tile_pool(name="w8", bufs=1) as w8_pool,
            tc.tile_pool(name="outp", bufs=10) as out_pool,
            tc.tile_pool(name="ps", bufs=2, space="PSUM") as ps_pool,
        ):
            wslabs = [
                w8_pool.tile([P, n_kt, 4 * P], fp8, name=f"wslab{ps}",
                             tag=f"wslab{ps}")
                for ps in range(n_pass)
            ]

            def emit_wchunk(ps, t0, nt):
                # nt k-tiles [t0, t0+nt) of this pass's o-range, contiguous
                # per partition, straight into the stationary slab
                dst = wslabs[ps][:, t0:t0 + nt, :]
                c0 = ps * W_PASS + t0 * 4 * P
                src = w_d[:, c0:c0 + nt * 4 * P]
                nc.scalar.dma_start(dst, src.rearrange("p (t o) -> p t o",
                                                       t=nt))

            def emit_xslab(xsup, u, g):
                # one packed pair slab: contiguous DMA, 2 KB/partition
                c0 = 2 * u * S_SUP
                nc.sync.dma_start(
                    xsup[:, g, :], x_d[g * P:(g + 1) * P, c0:c0 + 2 * S_SUP])

            def emit_xhalf(xsup, u, g, l):
                # half-slab (one s-chunk's worth): startup granularity
                c0 = 2 * u * S_SUP + 2 * l * N_CH
                nc.sync.dma_start(
                    xsup[:, g, 2 * l * N_CH:2 * (l + 1) * N_CH],
                    x_d[g * P:(g + 1) * P, c0:c0 + 2 * N_CH])

            def new_xsup():
                return xs_pool.tile([P, n_g, 2 * S_SUP], fp8, name="xsup",
                                    tag="xs")

            def rhs_ap(xsup, g, l):
                return xsup[:, g,
                            2 * l * N_CH:2 * (l + 1) * N_CH].rearrange(
                    "p (s j) -> p j s", j=2)

            def emit_evict(pst, ob, sc):
                # evict one bank to bf16 with the pow2 scale folded in;
                # store over the sync HWDGE
                ot = out_pool.tile([P, N_CH], bf16, name="ot", tag="ot")
                nc.vector.tensor_scalar(ot[:], pst[:], float(scale), None,
                                        AluOpType.mult)
                nc.sync.dma_start(
                    o_d[ob * P:(ob + 1) * P,
                        sc * N_CH:(sc + 1) * N_CH], ot[:])

            def bank_group(xsup, l, sc, ps, i):
                # accumulate one o-block over all k-groups, then evict
                pst = ps_pool.tile([P, N_CH], f32, name=f"ps{i}",
                                   tag=f"ps{i}")
                for g in range(n_g):
                    nc.tensor.matmul(
                        pst[:],
                        wslabs[ps][:, 2 * g:2 * g + 2, i * P:(i + 1) * P],
                        rhs_ap(xsup, g, l),
                        start=(g == 0), stop=(g == n_g - 1),
                        perf_mode=DR)
                emit_evict(pst, ps * 4 + i, sc)

            # --- startup: super 0's x goes out as 32 half-slab DMAs on
            # the sync queue; all w chunks go out pass-major on the
            # scalar queue.  The queues drain concurrently, so the first
            # matmul needs only the first x half-slab (128 KB) + the
            # first w half-chunk (128 KB) ---
            xsup0 = new_xsup()
            for g in range(n_g):
                for l in range(SUP):
                    emit_xhalf(xsup0, 0, g, l)
            emit_wchunk(0, 0, 2)  # split chunk 0: first matmul needs t 0..1
            emit_wchunk(0, 2, 2)
            for ps in range(n_pass):
                for mc in range(n_mc):
                    if ps == 0 and mc == 0:
                        continue
                    emit_wchunk(ps, mc * MC, MC)

            # (A PE warm-up against the p-state clock ramp was tried --
            # full-size DR matmuls on scratch during the DMA wait DO
            # pre-ramp the clock, tiny ones don't -- but it measurably
            # buys nothing: the ramp already overlaps the HBM-arrival-
            # limited first microseconds, and starting real matmuls at
            # full speed just converts ramp time into x-arrival stalls.)

            # --- super 0, pass 0: g-major across all 8 psum banks so PE
            # demand tracks the x half-slab arrival order (consumption
            # is 8 matmuls per half-slab; arrival is faster from t~=7us)
            psts0 = {}
            for l in range(SUP):
                for i in range(4):
                    psts0[(l, i)] = ps_pool.tile([P, N_CH], f32,
                                                 name=f"ps{i}", tag=f"ps{i}")
            for g in range(n_g):
                for l in range(SUP):
                    rhs = rhs_ap(xsup0, g, l)
                    for i in range(4):
                        nc.tensor.matmul(
                            psts0[(l, i)][:],
                            wslabs[0][:, 2 * g:2 * g + 2, i * P:(i + 1) * P],
                            rhs,
                            start=(g == 0), stop=(g == n_g - 1),
                            perf_mode=DR)
            for l in range(SUP):
                for i in range(4):
                    emit_evict(psts0[(l, i)], i, l)

            # --- super 0, passes 1..3: pass-major bank groups; super 1's
            # x slabs are emitted one per bank group, each AFTER that
            # group's evict+store ---
            xs_next = new_xsup() if n_sup > 1 else None
            pending_g = list(range(n_g)) if xs_next is not None else []

            def after_group(u):
                if pending_g:
                    emit_xslab(xs_next, u + 1, pending_g.pop(0))

            for ps in range(1, n_pass):
                for l in range(SUP):
                    for i in range(4):
                        bank_group(xsup0, l, l, ps, i)
                        after_group(0)

            # --- supers 1..: bank-major, one x prefetch slab per group
            xsup = xs_next
            for u in range(1, n_sup):
                while pending_g:  # catch up if super 0 underfed the list
                    after_group(u - 1)
                xs_next = new_xsup() if u + 1 < n_sup else None
                pending_g = list(range(n_g)) if xs_next is not None else []
                for l in range(SUP):
                    sc = u * SUP + l
                    for ps in range(n_pass):
                        for i in range(4):
                            bank_group(xsup, l, sc, ps, i)
                            after_group(u)
                while pending_g:
                    after_group(u)
                xsup = xs_next

    nc.compile()
    return nc


def _host_scale(shift_param):
    # np.round is round-half-to-even, matching jnp.round
    s = np.clip(np.float64(np.float32(shift_param)), -8.0, 0.0)
    return float(np.exp2(np.round(s)))


def kernel(x, weight, threshold, shift_param):
    import ml_dtypes

    from concourse.bass_utils import run_bass_kernel_spmd

    fp8 = ml_dtypes.float8_e4m3fn
    scale = _host_scale(shift_param)
    nc = build_program(scale=scale)

    # host prep: binarize to +/-1.0 fp8 via byte select (exact -- the
    # compare is the computation; 0x38/0xB8 are the e4m3 encodings of
    # +/-1.0) and lay out for contiguous device DMAs
    xf = np.asarray(x, np.float32).reshape(B * S, IN)
    x8 = np.where(xf >= 0.0, np.uint8(_FP8_ONE), np.uint8(_FP8_NEG_ONE))
    # pair-interleave transpose: [B*S, IN] -> [IN/2, 2*B*S], row c col
    # 2s+j = sign(x[s, 2c+j])
    x8 = np.ascontiguousarray(
        x8.reshape(B * S, IN // 2, 2).transpose(1, 0, 2)
        .reshape(IN // 2, 2 * B * S)).view(fp8)

    thr_f = np.asarray(threshold, np.float32).reshape(OUT, 1)
    d = np.asarray(weight, np.float32) - thr_f  # f32, exactly as reference
    w8 = np.where(d >= 0.0, np.uint8(_FP8_ONE), np.uint8(_FP8_NEG_ONE))

    n_pass = O_SHARD // (4 * P)  # 4
    n_g = IN // 256              # 16
    in_maps = []
    for c in range(N_CORES):
        wc = w8[c * O_SHARD:(c + 1) * O_SHARD]  # [2048, 4096] uint8
        # [o, k] -> [p, ps, g, j, oo]: o = ps*512 + oo, k = 256g + 2p + j
        wcs = wc.reshape(n_pass, 4 * P, n_g, P, 2).transpose(3, 0, 2, 4, 1)
        wcs = np.ascontiguousarray(wcs).reshape(P, -1).view(fp8)
        in_maps.append({"x": x8, "w": wcs})

    res = run_bass_kernel_spmd(nc, in_maps, list(range(N_CORES)),
                               trace=_TRACE)
    global _LAST_RESULTS
    _LAST_RESULTS = res
    shards = [res.results[c]["outT"] for c in range(N_CORES)]
    full_t = np.concatenate(shards, axis=0)  # [OUT, B*S]
    full = np.ascontiguousarray(full_t.astype(np.float32).T).reshape(B, S, OUT)
    return full
